# revision 7
# baseline (speedup 1.0000x reference)
"""Deformable Transformer Encoder Layer — Trainium2 Bass kernel (8 cores).

Sharding (hardcoded for the spec shapes): core c handles batch c//4 and a
6144-query slice (starts [0, 6144, 12288, 18421]; the last slice overlaps
11 rows so every slice is exactly 6144 queries).

Per core:
  Phase V: value = src_full @ w_val + b_val on PE (fp32), cast bf16 -> Vd,
    scatter into per-head zero-padded level grids G, then 4 shifted DRAM
    copies build Bt: per-head 2x2-pixel blocks (4*32 bf16 = 256B contiguous)
    so ONE 256B gather descriptor fetches all 4 bilinear corners of a sample.
  Phase Q (24 windows x 256 queries): q-side projections (PE), sampling
    location pipeline in (query, channel) layout on DVE (channel order
    c = h*16 + l*4 + p), global int32 block ids, per-(window, head)
    indirect-DMA gather of 4096 blocks, weight expansion on ACT, bf16
    multiply + binary-tree reduction on DVE -> attention output, then
    out-proj + LN + FFN + LN -> output.
"""

from contextlib import ExitStack

import numpy as np

import concourse.bass as bass
import concourse.tile as tile
from concourse import bacc, mybir
from concourse.bass import IndirectOffsetOnAxis
from concourse.bass_utils import run_bass_kernel_spmd

F32 = mybir.dt.float32
BF16 = mybir.dt.bfloat16
I32 = mybir.dt.int32
AF = mybir.ActivationFunctionType
ALU = mybir.AluOpType
AX = mybir.AxisListType

EMBED, HEADS, LEVELS, POINTS, FFN = 256, 8, 4, 4, 1024
SHAPES = [(136, 136), (68, 68), (34, 34), (17, 17)]
LSTART = [0, 18496, 23120, 24276]
NK = 24565
NKPAD = 24576
NT = NKPAD // 128
NQ = 6144
QSTARTS = [0, 6144, 12288, 18421]

GSTART = np.cumsum([0] + [(h + 2) * (w + 2) for h, w in SHAPES]).tolist()
GROWS = GSTART[-1]          # 25601 padded grid rows per head
BSTART = np.cumsum([0] + [(h + 1) * (w + 1) for h, w in SHAPES]).tolist()
BBLOCKS = BSTART[-1]        # 25079 blocks per head
NBLK = 8 * BBLOCKS

WQB = 2                     # 128-query blocks per window
WQ = 128 * WQB
NWIN_FULL = NQ // WQ        # 24
MAGIC = 12582912.0          # 1.5 * 2**23 fp32 round-to-nearest-int magic

_prog_cache = {}


def _channel_consts():
    l = (np.arange(128) // 4) % 4
    h = np.arange(128) // 16
    Wl = np.array([SHAPES[i][1] for i in range(4)], np.float32)
    Hl = np.array([SHAPES[i][0] for i in range(4)], np.float32)
    bs = np.array(BSTART[:4], np.float32)
    return {
        "sc_x": Wl[l], "sc_y": Hl[l],
        "lim_x": Wl[l] - 1.0, "lim_y": Hl[l] - 1.0,
        "AA": (Wl + 1.0)[l],
        "CC": (Wl + 1.0)[l] + 1.0 + bs[l] + h * float(BBLOCKS),
    }


def _tilec(row, reps=WQB):
    r = np.asarray(row, np.float32).reshape(1, 128)
    return np.ascontiguousarray(np.tile(r, (128, reps)))


def host_prep(inputs, core):
    b, ci = core // 4, core % 4
    qs = QSTARTS[ci]
    src = np.asarray(inputs["src"], np.float32)
    ref = np.asarray(inputs["reference_points"], np.float32)
    srcb = src[b]
    d = {}
    d["srcT"] = np.ascontiguousarray(
        np.pad(srcb, ((0, NKPAD - NK), (0, 0))).T)            # (256, 24576)
    d["srcTq"] = np.ascontiguousarray(d["srcT"][:, qs:qs + NQ])
    d["src_q"] = np.ascontiguousarray(srcb[qs:qs + NQ])
    lidx = (np.arange(128) // 4) % 4
    refq = ref[b, qs:qs + NQ]
    d["refx"] = np.ascontiguousarray(refq[:, lidx, 0])
    d["refy"] = np.ascontiguousarray(refq[:, lidx, 1])
    w_off = np.asarray(inputs["w_off"], np.float32)
    b_off = np.asarray(inputs["b_off"], np.float32)
    h = np.arange(128) // 16
    p = np.arange(128) % 4
    colx = h * 32 + lidx * 8 + p * 2
    d["w_offxy"] = np.ascontiguousarray(
        np.concatenate([w_off[:, colx], w_off[:, colx + 1]], axis=1))
    d["w_att"] = np.asarray(inputs["w_att"], np.float32)
    d["w_val"] = np.asarray(inputs["w_val"], np.float32)
    d["w_out"] = np.asarray(inputs["w_out"], np.float32)
    d["w1"] = np.asarray(inputs["w1"], np.float32)
    d["w2"] = np.asarray(inputs["w2"], np.float32)
    d["bias_rows"] = np.ascontiguousarray(np.stack([
        np.asarray(inputs["b_val"], np.float32),
        np.asarray(inputs["b_out"], np.float32),
        np.asarray(inputs["b2"], np.float32)]))               # (3, 256)
    d["b1t"] = np.ascontiguousarray(
        np.asarray(inputs["b1"], np.float32).reshape(8, 128).T)
    cc = _channel_consts()
    d["c_scx"] = _tilec(cc["sc_x"]); d["c_scy"] = _tilec(cc["sc_y"])
    d["c_limx"] = _tilec(cc["lim_x"]); d["c_limy"] = _tilec(cc["lim_y"])
    d["c_AA"] = _tilec(cc["AA"]); d["c_CC"] = _tilec(cc["CC"])
    d["c_bx"] = _tilec(b_off[colx] - 0.5)
    d["c_by"] = _tilec(b_off[colx + 1] - 0.5)
    d["c_batt"] = _tilec(np.asarray(inputs["b_att"], np.float32))
    for nm, key in (("c_g1", "g1"), ("c_bt1", "bt1"),
                    ("c_g2", "g2"), ("c_bt2", "bt2")):
        v = np.asarray(inputs[key], np.float32).reshape(1, 256)
        d[nm] = np.ascontiguousarray(np.tile(v, (128, 1)))
    d["eye"] = np.eye(128, dtype=np.float32)
    d["ones1"] = np.ones((1, 128), np.float32)
    return d


INPUT_SPECS = [
    ("srcT", (256, NKPAD)), ("srcTq", (256, NQ)), ("src_q", (NQ, 256)),
    ("refx", (NQ, 128)), ("refy", (NQ, 128)),
    ("w_offxy", (256, 256)), ("w_att", (256, 128)),
    ("w_val", (256, 256)), ("w_out", (256, 256)),
    ("w1", (256, FFN)), ("w2", (FFN, 256)),
    ("bias_rows", (3, 256)), ("b1t", (128, 8)),
    ("c_scx", (128, WQ)), ("c_scy", (128, WQ)),
    ("c_limx", (128, WQ)), ("c_limy", (128, WQ)),
    ("c_AA", (128, WQ)), ("c_CC", (128, WQ)),
    ("c_bx", (128, WQ)), ("c_by", (128, WQ)), ("c_batt", (128, WQ)),
    ("c_g1", (128, 256)), ("c_bt1", (128, 256)),
    ("c_g2", (128, 256)), ("c_bt2", (128, 256)),
    ("eye", (128, 128)), ("ones1", (1, 128)),
]


def build_kernel_body(tc, ins, out, Vd, G, Bt, n_windows):
    nc = tc.nc
    P = 128
    ctx = ExitStack()
    ctx.__enter__()

    const = ctx.enter_context(tc.tile_pool(name="const", bufs=1))

    def load_small(name):
        src = ins[name]
        t = const.tile(list(src.shape), F32, name=f"cst_{name}",
                       tag=f"cst_{name}")
        nc.sync.dma_start(t, src)
        return t

    def load_chunked(name, rows, cols):
        n = rows // 128
        t = const.tile([128, n, cols], F32, name=f"ck_{name}",
                       tag=f"ck_{name}")
        for i in range(n):
            nc.sync.dma_start(t[:, i, :], ins[name][i * 128:(i + 1) * 128, :])
        return t

    bias_row = []
    for i in range(3):
        brt = const.tile([1, 256], F32, name=f"brow{i}", tag=f"brow{i}")
        nc.sync.dma_start(brt, ins["bias_rows"][i:i + 1, :])
        bias_row.append(brt)
    b1t = load_small("b1t")
    eye = load_small("eye")
    ones1 = load_small("ones1")
    C = {k: load_small(k) for k in (
        "c_scx", "c_scy", "c_limx", "c_limy", "c_AA", "c_CC",
        "c_bx", "c_by", "c_batt", "c_g1", "c_bt1", "c_g2", "c_bt2")}
    w_val_c = load_chunked("w_val", 256, 256)
    w_offxy_c = load_chunked("w_offxy", 256, 256)
    w_att_c = load_chunked("w_att", 256, 128)
    w_out_c = load_chunked("w_out", 256, 256)
    w1_c = load_chunked("w1", 256, FFN)
    w2_c = load_chunked("w2", FFN, 256)

    psum = ctx.enter_context(tc.tile_pool(name="psum", bufs=6, space="PSUM"))

    def psum_tile(name):
        return psum.tile([P, 512], F32, name=name, tag="ps")

    # ---------------- Phase V ----------------
    lhsp = ctx.enter_context(tc.tile_pool(name="lhsp", bufs=4))
    vtp = ctx.enter_context(tc.tile_pool(name="vtp", bufs=3))
    for t in range(NT):
        lhs = lhsp.tile([P, 2, P], F32, name="lhs", tag="lhs")
        for kc in range(2):
            nc.sync.dma_start(
                lhs[:, kc, :],
                ins["srcT"][kc * 128:(kc + 1) * 128, t * 128:(t + 1) * 128])
        pv = psum_tile("pv")
        nc.tensor.matmul(pv[:, 0:256], lhs[:, 0, :], w_val_c[:, 0, :],
                         start=True, stop=False)
        nc.tensor.matmul(pv[:, 0:256], lhs[:, 1, :], w_val_c[:, 1, :],
                         start=False, stop=False)
        nc.tensor.matmul(pv[:, 0:256], ones1, bias_row[0],
                         start=False, stop=True)
        vt = vtp.tile([P, 256], BF16, name="vt", tag="vt")
        nc.scalar.activation(vt, pv[:, 0:256], AF.Copy)
        nc.sync.dma_start(Vd[t * 128:(t + 1) * 128, :], vt)

    # zero G (pads matter; interior gets overwritten)
    zp = ctx.enter_context(tc.tile_pool(name="zp", bufs=1))
    zt = zp.tile([P, 2048], BF16, name="zt", tag="zt")
    nc.vector.memset(zt, 0.0)
    gflat = G.rearrange("h r d -> (h r d)")
    total = 8 * GROWS * 32
    off = 0
    while off < total:
        rows = min((total - off) // 2048, P)
        if rows >= 1:
            nc.sync.dma_start(
                gflat[off:off + rows * 2048].rearrange("(p x) -> p x", p=rows),
                zt[0:rows, :])
            off += rows * 2048
        else:
            rem = total - off
            nc.sync.dma_start(
                gflat[off:off + rem].rearrange("(p x) -> p x", p=1),
                zt[0:1, 0:rem])
            off = total

    # Vd -> G interiors
    for h in range(8):
        for li, (H, W) in enumerate(SHAPES):
            src = Vd[LSTART[li]:LSTART[li] + H * W, h * 32:(h + 1) * 32]
            src = src.rearrange("(y x) d -> y x d", x=W)
            dst = G[h, GSTART[li]:GSTART[li + 1], :].rearrange(
                "(y x) d -> y x d", x=W + 2)
            nc.sync.dma_start(dst[1:H + 1, 1:W + 1, :], src)

    # G -> Bt (4 shifted copies per head per level)
    for h in range(8):
        for li, (H, W) in enumerate(SHAPES):
            g = G[h, GSTART[li]:GSTART[li + 1], :].rearrange(
                "(y x) d -> y x d", x=W + 2)
            bb = Bt[h * BBLOCKS + BSTART[li]:
                    h * BBLOCKS + BSTART[li + 1], :].rearrange(
                "(y x) (c d) -> y x c d", x=W + 1, c=4)
            for c, (dy, dx) in enumerate(((0, 0), (0, 1), (1, 0), (1, 1))):
                nc.sync.dma_start(bb[:, :, c, :],
                                  g[dy:dy + H + 1, dx:dx + W + 1, :])

    # ---------------- Phase Q ----------------
    wp = ctx.enter_context(tc.tile_pool(name="wp", bufs=2))
    pp = ctx.enter_context(tc.tile_pool(name="pp", bufs=1))
    gp = ctx.enter_context(tc.tile_pool(name="gp", bufs=2))
    trp = ctx.enter_context(tc.tile_pool(name="trp", bufs=1))
    ep = ctx.enter_context(tc.tile_pool(name="ep", bufs=2))

    def ptile(name, shape=None, dt=F32, pool=pp, bufs=None):
        return pool.tile([P, WQ] if shape is None else list(shape), dt,
                         name=name, tag=name, bufs=bufs)

    def tt(name, a, b, op, shape=None, dt=F32, pool=pp):
        o = ptile(name, shape=shape, dt=dt, pool=pool)
        nc.vector.tensor_tensor(out=o, in0=a, in1=b, op=op)
        return o

    def layernorm(x_ap, gtile, btile, tag):
        sc = ptile(f"ln_sc", shape=[P, 256])
        ssq = ptile(f"ln_ssq_{tag}", shape=[P, 1])
        nc.scalar.activation(sc, x_ap, AF.Square, accum_out=ssq)
        s1 = ptile(f"ln_s1_{tag}", shape=[P, 1])
        nc.vector.reduce_sum(out=s1, in_=x_ap, axis=AX.X)
        m = ptile(f"ln_m_{tag}", shape=[P, 1])
        nc.vector.tensor_scalar_mul(out=m, in0=s1, scalar1=1.0 / 256)
        q2 = ptile(f"ln_q2_{tag}", shape=[P, 1])
        nc.vector.tensor_scalar_mul(out=q2, in0=ssq, scalar1=1.0 / 256)
        m2 = ptile(f"ln_m2_{tag}", shape=[P, 1])
        nc.vector.tensor_tensor(out=m2, in0=m, in1=m, op=ALU.mult)
        var = ptile(f"ln_var_{tag}", shape=[P, 1])
        nc.vector.tensor_tensor(out=var, in0=q2, in1=m2, op=ALU.subtract)
        vare = ptile(f"ln_ve_{tag}", shape=[P, 1])
        nc.vector.tensor_scalar_add(out=vare, in0=var, scalar1=1e-5)
        sd = ptile(f"ln_sd_{tag}", shape=[P, 1])
        nc.scalar.activation(sd, vare, AF.Sqrt)
        rstd = ptile(f"ln_rs_{tag}", shape=[P, 1])
        nc.vector.reciprocal(rstd, sd)
        xn = ptile(f"ln_xn_{tag}", shape=[P, 256])
        nc.vector.tensor_scalar(out=xn, in0=x_ap, scalar1=m,
                                scalar2=None, op0=ALU.subtract)
        y = ptile(f"ln_y_{tag}", shape=[P, 256])
        nc.vector.scalar_tensor_tensor(out=y, in0=xn, scalar=rstd,
                                       in1=gtile, op0=ALU.mult, op1=ALU.mult)
        yo = ptile(f"ln_o_{tag}", shape=[P, 256])
        nc.vector.tensor_tensor(out=yo, in0=y, in1=btile, op=ALU.add)
        return yo

    for w in range(n_windows):
        q0 = w * WQ
        srcw = wp.tile([P, 2, WQ], F32, name="srcw", tag="srcw")
        for kc in range(2):
            nc.sync.dma_start(
                srcw[:, kc, :],
                ins["srcTq"][kc * 128:(kc + 1) * 128, q0:q0 + WQ])
        refxw = wp.tile([P, WQ], F32, name="refxw", tag="refxw")
        nc.sync.dma_start(
            refxw.rearrange("p (qb c) -> p qb c", qb=WQB),
            ins["refx"][q0:q0 + WQ, :].rearrange("(qb p) c -> p qb c", p=128))
        refyw = wp.tile([P, WQ], F32, name="refyw", tag="refyw")
        nc.sync.dma_start(
            refyw.rearrange("p (qb c) -> p qb c", qb=WQB),
            ins["refy"][q0:q0 + WQ, :].rearrange("(qb p) c -> p qb c", p=128))

        p_off = psum_tile("p_off")
        p_att = psum_tile("p_att")
        for qb in range(WQB):
            qsl = slice(qb * 128, qb * 128 + 128)
            for kc in range(2):
                nc.tensor.matmul(p_off[:, qb * 256:qb * 256 + 256],
                                 srcw[:, kc, qsl], w_offxy_c[:, kc, :],
                                 start=(kc == 0), stop=(kc == 1))
            for kc in range(2):
                nc.tensor.matmul(p_att[:, qsl],
                                 srcw[:, kc, qsl], w_att_c[:, kc, :],
                                 start=(kc == 0), stop=(kc == 1))

        offv = p_off.rearrange("p (qb two c) -> p qb two c", qb=WQB, two=2)
        offx = offv[:, :, 0, :]
        offy = offv[:, :, 1, :]

        def coord_pipe(refw, offsl, cbias, csc, clim, pre):
            t0 = tt(f"{pre}_t0", refw, csc, ALU.mult)
            t1 = tt(f"{pre}_t1",
                    t0.rearrange("p (qb c) -> p qb c", qb=WQB), offsl,
                    ALU.add)
            pxy = tt(f"{pre}_px", t1, cbias, ALU.add)
            f = ptile(f"{pre}_f")
            nc.vector.tensor_scalar(out=f, in0=pxy, scalar1=MAGIC,
                                    scalar2=-MAGIC, op0=ALU.add, op1=ALU.add)
            gt = tt(f"{pre}_gt", f, pxy, ALU.is_gt)
            v0 = tt(f"{pre}_v0", f, gt, ALU.subtract)
            fr = tt(f"{pre}_fr", pxy, v0, ALU.subtract)
            vm = ptile(f"{pre}_vm")
            nc.vector.tensor_scalar_max(out=vm, in0=v0, scalar1=-1.0)
            vc = tt(f"{pre}_vc", vm, clim, ALU.min)
            mk = tt(f"{pre}_mk", vc, v0, ALU.is_equal)
            om = ptile(f"{pre}_om")
            nc.vector.tensor_scalar(out=om, in0=fr, scalar1=-1.0,
                                    scalar2=1.0, op0=ALU.mult, op1=ALU.add)
            w0 = tt(f"{pre}_w0", om, mk, ALU.mult)
            w1_ = tt(f"{pre}_w1", fr, mk, ALU.mult)
            return vc, w0, w1_

        xc, wx0, wx1 = coord_pipe(refxw, offx, C["c_bx"], C["c_scx"],
                                  C["c_limx"], "x")
        yc, wy0, wy1 = coord_pipe(refyw, offy, C["c_by"], C["c_scy"],
                                  C["c_limy"], "y")

        ea = tt("ea", p_att[:, 0:WQ], C["c_batt"], ALU.add)
        e = ptile("e")
        nc.scalar.activation(e, ea, AF.Exp)
        s16 = ptile("s16", shape=[P, WQB * 8])
        nc.vector.reduce_sum(
            out=s16, in_=e.rearrange("p (qb h l) -> p qb h l", qb=WQB, h=8),
            axis=AX.X)
        r16 = ptile("r16", shape=[P, WQB * 8])
        nc.vector.reciprocal(r16, s16)
        aw = ptile("aw")
        nc.vector.tensor_tensor(
            out=aw.rearrange("p (qb h l) -> p qb h l", qb=WQB, h=8),
            in0=e.rearrange("p (qb h l) -> p qb h l", qb=WQB, h=8),
            in1=r16.rearrange("p (qb h) -> p qb h", qb=WQB)
                .unsqueeze(3).broadcast_to([P, WQB, 8, 16]),
            op=ALU.mult)
        wy0a = tt("wy0a", wy0, aw, ALU.mult)
        wy1a = tt("wy1a", wy1, aw, ALU.mult)

        w4 = pp.tile([P, 4, WQ], F32, name="w4", tag="w4", bufs=2)
        nc.vector.tensor_tensor(out=w4[:, 0, :], in0=wy0a, in1=wx0,
                                op=ALU.mult)
        nc.vector.tensor_tensor(out=w4[:, 1, :], in0=wy0a, in1=wx1,
                                op=ALU.mult)
        nc.vector.tensor_tensor(out=w4[:, 2, :], in0=wy1a, in1=wx0,
                                op=ALU.mult)
        nc.vector.tensor_tensor(out=w4[:, 3, :], in0=wy1a, in1=wx1,
                                op=ALU.mult)

        blk_a = tt("blk_a", yc, C["c_AA"], ALU.mult)
        blk_b = tt("blk_b", blk_a, xc, ALU.add)
        blkf = tt("blk_c", blk_b, C["c_CC"], ALU.add)
        blk32 = pp.tile([P, WQ], I32, name="blk32", tag="blk32", bufs=2)
        nc.vector.tensor_copy(out=blk32, in_=blkf)

        ac = gp.tile([P, WQB, 256], F32, name="ac", tag="ac")
        for h in range(8):
            Tw = gp.tile([P, WQB * 16, 128], BF16, name="Tw", tag="Tw")
            blkv = blk32.rearrange("p (qb c) -> p qb c",
                                   qb=WQB)[:, :, h * 16:h * 16 + 16]
            nc.gpsimd.indirect_dma_start(
                out=Tw, out_offset=None,
                in_=Bt, in_offset=IndirectOffsetOnAxis(ap=blkv, axis=0))
            wexp = gp.tile([P, WQB * 2048], BF16, name="wexp", tag="wexp")
            wv = w4.rearrange("p c (qb x) -> p qb x c", qb=WQB)
            for qb in range(WQB):
                wvh = wv[:, qb, h * 16:h * 16 + 16, :]
                nc.scalar.activation(
                    wexp[:, qb * 2048:(qb + 1) * 2048].rearrange(
                        "p (l c d) -> p l c d", l=16, c=4),
                    wvh.unsqueeze(3).broadcast_to([P, 16, 4, 32]),
                    AF.Copy)
            pr = gp.tile([P, WQB * 2048], BF16, name="pr", tag="pr")
            nc.vector.tensor_tensor(
                out=pr, in0=Tw.rearrange("p a b -> p (a b)"),
                in1=wexp, op=ALU.mult)
            lvl = pr.rearrange("p (qb l x) -> p qb l x", qb=WQB, l=16)
            nlp = 16
            for li in range(4):
                nxt = trp.tile([P, WQB, nlp // 2, 128], BF16,
                               name=f"tr{li}", tag=f"tr{li}")
                nc.vector.tensor_tensor(
                    out=nxt, in0=lvl[:, :, 0:nlp // 2, :],
                    in1=lvl[:, :, nlp // 2:nlp, :], op=ALU.add)
                lvl = nxt
                nlp //= 2
            l4 = lvl.rearrange("p qb one (c d) -> p qb (one c) d", c=4)
            l5 = trp.tile([P, WQB, 2, 32], BF16, name="tr4", tag="tr4")
            nc.vector.tensor_tensor(out=l5, in0=l4[:, :, 0:2, :],
                                    in1=l4[:, :, 2:4, :], op=ALU.add)
            nc.vector.tensor_tensor(
                out=ac[:, :, h * 32:h * 32 + 32],
                in0=l5[:, :, 0, :], in1=l5[:, :, 1, :], op=ALU.add)

        # epilogue
        xw = ep.tile([P, WQB, 256], F32, name="xw", tag="xw")
        xTw = ep.tile([P, 2, WQ], F32, name="xTw", tag="xTw")
        for qb in range(WQB):
            p_t = psum_tile("p_t")
            for kc in range(2):
                nc.tensor.transpose(p_t[:, kc * 128:kc * 128 + 128],
                                    ac[:, qb, kc * 128:kc * 128 + 128], eye)
            acT = ep.tile([P, 256], F32, name="acT", tag="acT")
            nc.scalar.activation(acT, p_t[:, 0:256], AF.Copy)
            p_y = psum_tile("p_y")
            for kc in range(2):
                nc.tensor.matmul(p_y[:, 0:256],
                                 acT[:, kc * 128:kc * 128 + 128],
                                 w_out_c[:, kc, :], start=(kc == 0),
                                 stop=False)
            nc.tensor.matmul(p_y[:, 0:256], ones1, bias_row[1],
                             start=False, stop=True)
            sqt = ep.tile([P, 256], F32, name="sqt", tag="sqt")
            nc.sync.dma_start(
                sqt, ins["src_q"][q0 + qb * 128:q0 + qb * 128 + 128, :])
            x0t = ep.tile([P, 256], F32, name="x0t", tag="x0t")
            nc.vector.tensor_tensor(out=x0t, in0=sqt, in1=p_y[:, 0:256],
                                    op=ALU.add)
            xln = layernorm(x0t, C["c_g1"], C["c_bt1"], f"a{qb}")
            nc.vector.tensor_copy(out=xw[:, qb, :], in_=xln)
            p_t2 = psum_tile("p_t2")
            for kc in range(2):
                nc.tensor.transpose(p_t2[:, kc * 128:kc * 128 + 128],
                                    xln[:, kc * 128:kc * 128 + 128], eye)
            nc.scalar.activation(
                xTw[:, :, qb * 128:qb * 128 + 128],
                p_t2[:, 0:256].rearrange("p (k q) -> p k q", k=2),
                AF.Copy)

        hW = ep.tile([P, 8, WQ], F32, name="hW", tag="hW", bufs=1)
        for oc in range(8):
            p_h = psum_tile("p_h")
            for kc in range(2):
                nc.tensor.matmul(p_h[:, 0:WQ],
                                 w1_c[:, kc, oc * 128:oc * 128 + 128],
                                 xTw[:, kc, :], start=(kc == 0),
                                 stop=(kc == 1))
            nc.scalar.activation(hW[:, oc, :], p_h[:, 0:WQ], AF.Relu,
                                 bias=b1t[:, oc:oc + 1])

        for qb in range(WQB):
            p_f = psum_tile("p_f")
            for oc in range(8):
                nc.tensor.matmul(p_f[:, 0:256],
                                 hW[:, oc, qb * 128:qb * 128 + 128],
                                 w2_c[:, oc, :], start=(oc == 0), stop=False)
            nc.tensor.matmul(p_f[:, 0:256], ones1, bias_row[2],
                             start=False, stop=True)
            x2 = ep.tile([P, 256], F32, name="x2", tag="x2")
            nc.vector.tensor_tensor(out=x2, in0=xw[:, qb, :],
                                    in1=p_f[:, 0:256], op=ALU.add)
            outt = layernorm(x2, C["c_g2"], C["c_bt2"], f"b{qb}")
            nc.sync.dma_start(
                out[q0 + qb * 128:q0 + qb * 128 + 128, :], outt)

    ctx.__exit__(None, None, None)


def build_program(n_windows=NWIN_FULL):
    key = n_windows
    if key in _prog_cache:
        return _prog_cache[key]
    nc = bacc.Bacc("TRN2", target_bir_lowering=False, debug=False,
                   enable_asserts=False, num_devices=8)
    ins = {}
    for name, shape in INPUT_SPECS:
        ins[name] = nc.dram_tensor(name, list(shape), F32,
                                   kind="ExternalInput").ap()
    out = nc.dram_tensor("out", [NQ, 256], F32, kind="ExternalOutput").ap()
    Vd = nc.dram_tensor("Vd", [NKPAD, 256], BF16).ap()
    G = nc.dram_tensor("G", [8, GROWS, 32], BF16).ap()
    Bt = nc.dram_tensor("Bt", [NBLK, 128], BF16).ap()
    with tile.TileContext(nc) as tc:
        build_kernel_body(tc, ins, out, Vd, G, Bt, n_windows)
    nc.compile()
    _prog_cache[key] = (nc, [n for n, _ in INPUT_SPECS])
    return _prog_cache[key]


def kernel(**inputs) -> np.ndarray:
    nc, names = build_program()
    in_maps = []
    for core in range(8):
        d = host_prep(inputs, core)
        in_maps.append({n: np.ascontiguousarray(d[n], np.float32)
                        for n in names})
    res = run_bass_kernel_spmd(nc, in_maps, list(range(8)))
    out = np.zeros((2, NK, EMBED), np.float32)
    for core in range(8):
        b, ci = core // 4, core % 4
        r = np.asarray(res.results[core]["out"])
        if ci < 3:
            out[b, QSTARTS[ci]:QSTARTS[ci] + NQ] = r
        else:
            out[b, 18432:NK] = r[11:]
    return out
